# revision 45
# baseline (speedup 1.0000x reference)
"""Trainium2 Bass kernel for MoEAdaptorLayer (moe_routing).

Reference computation (B=512, L=50, D=768, O=300, E=8):
    gates = softmax(x @ w_gate)                          # [B,L,E]
    xw    = einsum('bli,eoi->bleo', x, expert_w)         # [B,L,E,O]
    bw    = einsum('eli,eoi->leo', expert_bias, expert_w)
    out   = einsum('ble,bleo->blo', gates, xw - bw[None])

Strategy: data-parallel over B across 8 cores (64 batches/core); no
collectives. Tokens are laid out l-major per core (token = l*64 + b), so each
128-token tile covers exactly two l values. Matmul operands are fp16 (fp32
PSUM accumulation); negbw[e,l,o] = -sum_i bias*W is precomputed on the host
(weight-only preprocessing, 0.2% of total FLOPs) and pre-replicated at
partition offsets 0/32/64/96. Per 128-token tile, emission is expert-major so
each expert's PSUM bank closes as early as possible:
  - e0's matmul packs the gate-logit columns; as soon as its bank closes, ACT
    computes exp-gates ghat (+row-sum), DVE the reciprocal and normalized gn.
  - DVE's 32x32 block-transpose produces ghat^T per token quarter; the
    gate-weighted bias correction runs as four concurrent diagonal 32x32 PE
    tiles (K=8, M=32) into the corr bank.
  - the weighted sum over experts is split: ACT does scaled copies
    t_e = gn_e*P_e for e0-e3 (freeing those banks early) plus acci = rs*corr;
    DVE folds e4-e6 via scalar_tensor_tensor and merges with an fp16 tree
    while e5/e6/e7 stream, so a single STT (which also folds e7) remains
    after each tile's last matmul.
Startup: tile 0 runs chunk-major so the PE consumes each w chunk as its DMA
lands; w rides the Scalar engine's hardware DGE queue while x/out use Sync's;
negbw loads in column blocks ordered around the w chunks.
"""

import sys

sys.path.insert(0, "/opt/trn_rl_repo")

from contextlib import ExitStack

import numpy as np

import concourse.bass as bass  # noqa: F401
import concourse.tile as tile
from concourse import bacc, mybir
from concourse import bass_utils

B, L, D, O, E = 512, 50, 768, 300, 8
NCORES = 8
BC = B // NCORES          # 64 batches per core
TOK = BC * L              # 3200 tokens per core
P = 128                   # tokens per tile
NT = TOK // P             # 25 tiles per core
KC = D // 128             # 6 contraction chunks
WCOL = E + E * O          # packed w row: [gate(8) | e0(300) | ... | e7(300)]
PTW = O                   # pcor tile width (corr accumulator)

F32 = mybir.dt.float32
FP16 = mybir.dt.float16

_CACHE: dict = {}


def _build_nc():
    nc = bacc.Bacc("TRN2", target_bir_lowering=False, debug=False,
                   num_devices=NCORES)

    xt_d = nc.dram_tensor("xt", [NT, P, KC, 128], FP16, kind="ExternalInput").ap()
    w_d = nc.dram_tensor("w", [128, KC, WCOL], FP16, kind="ExternalInput").ap()
    # negbw pre-replicated on host at partition offsets 0/32/64/96 (the corr
    # matmuls run as four diagonal 32x32 array tiles, one per token quarter).
    # Full-width rows: partition-sparse DMAs run at ~1/16 bandwidth, so the
    # 72 junk rows are cheaper than a compact transfer.
    nbw_d = nc.dram_tensor("nbw", [104, L * O], FP16, kind="ExternalInput").ap()
    out_d = nc.dram_tensor("out", [NT, P, O], FP16, kind="ExternalOutput").ap()

    AF = mybir.ActivationFunctionType
    ALU = mybir.AluOpType

    with tile.TileContext(nc) as tc, ExitStack() as ctx:
        const = ctx.enter_context(tc.tile_pool(name="const", bufs=1))
        xpool = ctx.enter_context(tc.tile_pool(name="xpool", bufs=3))
        spool = ctx.enter_context(tc.tile_pool(name="spool", bufs=3))
        tpool = ctx.enter_context(tc.tile_pool(name="tpool", bufs=8))
        apool = ctx.enter_context(tc.tile_pool(name="apool", bufs=14))
        opool = ctx.enter_context(tc.tile_pool(name="opool", bufs=3))
        pexp = ctx.enter_context(tc.tile_pool(name="pexp", bufs=7, space="PSUM"))
        pcor = ctx.enter_context(tc.tile_pool(name="pcor", bufs=1, space="PSUM"))

        # --- constants: params pre-packed on host, fp16 -------------------
        # Two hardware DGE queues exist (Sync and Scalar engines); the w
        # chunks alternate between them so the startup load runs at
        # aggregate HBM bandwidth. x tiles + outputs stay on Sync.
        xr0 = xpool.tile([P, KC, 128], FP16, tag="xr", name="xr_pre0")
        nc.sync.dma_start(xr0[:], xt_d[0])

        # negbw loads in column blocks ordered around the w chunks: tile 0's
        # block lands before w0, the rest after w5 — so the 4x replication
        # never delays the critical w load. Four slice-DMAs per block put
        # the copies at partition offsets 0/32/64/96.
        negbw = const.tile([104, L * O], FP16, tag="negbw")
        # column-block boundaries: tile 0's l-pair before w0, rest after w5
        nbnd = [0, 2 * O, 4200, 7800, 11400, L * O]

        # w loads in two expert-halves: [gates+e0-e3 of every chunk] first,
        # then [e4-e7]. Tile 0's first half streams during the A-half load,
        # its gate/corr path fires at the halfway point, and its second half
        # plus tile 1's first half keep the PE saturated during the B-half.
        WSPL = E + 4 * O
        w_sb = []
        for c in range(KC):
            wc = const.tile([128, WCOL], FP16, tag=f"w{c}", name=f"w_sb{c}")
            nc.scalar.dma_start(wc[:, 0:WSPL], w_d[:, c, 0:WSPL])
            w_sb.append(wc)
        # tile 0's corr columns + x tiles 1-2 land between the halves (the
        # Sync queue carries only x0 during the load, so w gets full HBM)
        nc.scalar.dma_start(negbw[:, 0:nbnd[1]], nbw_d[:, 0:nbnd[1]])
        xr1 = xpool.tile([P, KC, 128], FP16, tag="xr", name="xr_pre1")
        nc.scalar.dma_start(xr1[:], xt_d[1])
        xr2 = xpool.tile([P, KC, 128], FP16, tag="xr", name="xr_pre2")
        nc.scalar.dma_start(xr2[:], xt_d[2])
        for c in range(KC):
            nc.scalar.dma_start(w_sb[c][:, WSPL:WCOL], w_d[:, c, WSPL:WCOL])
        for blk in range(1, 5):
            cs = slice(nbnd[blk], nbnd[blk + 1])
            nc.scalar.dma_start(negbw[:, cs], nbw_d[:, cs])

        # --- token tiles --------------------------------------------------
        def alloc_pes(t):
            return [pexp.tile([P, E + O if e == 0 else O], F32, tag="pexp",
                              name=f"pe{t}_{e}") for e in range(E)]

        def emit_tile(t, xr=None, pes_pre=None, skip_e012=False,
                      g1_hook=None):
            if xr is None:
                xr = xpool.tile([P, KC, 128], FP16, tag="xr", name=f"xr{t}")
                nc.sync.dma_start(xr[:], xt_d[t])

            pes = pes_pre if pes_pre is not None else alloc_pes(t)
            ptr = pcor.tile([P, PTW], F32, tag="pcor", name=f"pc{t}")

            ghat = spool.tile([P, 32], FP16, tag="ghat", name=f"ghat{t}")
            gsum = spool.tile([P, 1], F32, tag="gsum", name=f"gsum{t}")
            rs = spool.tile([P, 1], F32, tag="rs", name=f"rs{t}")
            gn = spool.tile([P, E], F32, tag="gn", name=f"gn{t}")
            gtT = spool.tile([P, 32], FP16, tag="gtT", name=f"gtT{t}")
            acci = spool.tile([P, O], FP16, tag="acci", name=f"acci{t}")
            ts = [tpool.tile([P, O], FP16, tag="tmp", name=f"t{t}_{e}")
                  for e in range(4)]
            a4 = apool.tile([P, O], FP16, tag="stt", name=f"a4_{t}")
            a5 = apool.tile([P, O], FP16, tag="stt", name=f"a5_{t}")
            a6 = apool.tile([P, O], FP16, tag="stt", name=f"a6_{t}")
            m = apool.tile([P, O], FP16, tag="stt", name=f"m_{t}")
            z1 = apool.tile([P, O], FP16, tag="stt", name=f"z1_{t}")
            z2 = apool.tile([P, O], FP16, tag="stt", name=f"z2_{t}")
            z3 = apool.tile([P, O], FP16, tag="stt", name=f"z3_{t}")
            osb = opool.tile([P, O], FP16, tag="osb", name=f"osb{t}")

            def mm(e, c):
                lo = 0 if e == 0 else E + e * O
                nc.tensor.matmul(pes[e][:], xr[:, c, :],
                                 w_sb[c][:, lo:lo + pes[e].shape[-1]],
                                 start=(c == 0), stop=(c == KC - 1))

            def mm_expert(e):
                for c in range(KC):
                    mm(e, c)

            # Gate path + corr. The gate transpose runs on the DVE as a
            # 32x32 block transpose: gtT block b holds ghat^T for tokens
            # 32b..32b+32 on partitions 32b..32b+8 (cols 8:32 of ghat are
            # filled with exp of expert-0 columns purely so the transpose
            # input is fully defined; those partitions are never read).
            def emit_gates():
                # normalized gates: ghat = exp(logits), gn = ghat/sum
                nc.scalar.activation(ghat[:, 0:E], pes[0][:, 0:E], AF.Exp,
                                     accum_out=gsum[:])
                nc.scalar.activation(ghat[:, E:32], pes[0][:, E:32], AF.Exp)
                nc.vector.reciprocal(rs[:], gsum[:])
                nc.vector.tensor_scalar_mul(gn[:], ghat[:, 0:E], rs[:])
                nc.vector.transpose(gtT[:], ghat[:])

            def emit_corr():
                # gate-weighted bias correction as four concurrent
                # diagonal 32x32 PE tiles (K=8, M=32 each):
                # corr[m,:] = sum_e ghat[m,e] * negbw[l(m),e,:]  (unnorm.)
                for b in range(4):
                    lt = 2 * t + (b // 2)
                    nc.tensor.matmul(ptr[32 * b:32 * (b + 1), 0:O],
                                     gtT[32 * b:32 * b + E, 0:32],
                                     negbw[32 * b:32 * b + E,
                                           lt * O:(lt + 1) * O],
                                     start=True, stop=True,
                                     skip_group_check=True,
                                     tile_position=(32 * b, 32 * b))

            def emit_tail():
                # weighted sum: ACT drained e0-e3 into ts[*]; DVE folds
                # e4-e6 and the corr into a merge tree while e5/e6/e7
                # stream, leaving a single STT after the last matmul
                nc.vector.scalar_tensor_tensor(a4[:], pes[4][:], gn[:, 4:5],
                                               ts[1][:], op0=ALU.mult,
                                               op1=ALU.add)
                nc.scalar.mul(acci[:], ptr[:, 0:O], rs[:])
                if t != 0:
                    mm_expert(5)
                nc.vector.scalar_tensor_tensor(a5[:], pes[5][:], gn[:, 5:6],
                                               ts[2][:], op0=ALU.mult,
                                               op1=ALU.add)
                nc.vector.tensor_add(m[:], ts[0][:], acci[:])
                nc.vector.tensor_add(z1[:], a4[:], a5[:])
                if t != 0:
                    mm_expert(6)
                nc.vector.scalar_tensor_tensor(a6[:], pes[6][:], gn[:, 6:7],
                                               ts[3][:], op0=ALU.mult,
                                               op1=ALU.add)
                nc.vector.tensor_add(z2[:], a6[:], m[:])
                nc.vector.tensor_add(z3[:], z1[:], z2[:])
                if t != 0:
                    mm_expert(7)
                if t == NT - 1:
                    # split the final fold so the out DMA starts earlier
                    half = O // 2
                    nc.vector.scalar_tensor_tensor(
                        osb[:, 0:half], pes[7][:, 0:half], gn[:, 7:8],
                        z3[:, 0:half], op0=ALU.mult, op1=ALU.add)
                    nc.sync.dma_start(out_d[t][:, 0:half], osb[:, 0:half])
                    nc.vector.scalar_tensor_tensor(
                        osb[:, half:O], pes[7][:, half:O], gn[:, 7:8],
                        z3[:, half:O], op0=ALU.mult, op1=ALU.add)
                    nc.sync.dma_start(out_d[t][:, half:O], osb[:, half:O])
                else:
                    nc.vector.scalar_tensor_tensor(osb[:], pes[7][:],
                                                   gn[:, 7:8], z3[:],
                                                   op0=ALU.mult, op1=ALU.add)
                    nc.sync.dma_start(out_d[t], osb[:])

            if t == 0:
                # startup tile: chunk-major per expert-half so the PE
                # streams each half-chunk DMA as it lands; the gate path
                # fires at the halfway point and the ACT drain of e0-e3
                # frees banks for the second half and tile 1
                for c in range(KC):
                    for e in range(4):
                        mm(e, c)
                emit_gates()
                nc.scalar.mul(ts[0][:], pes[0][:, E:E + O], gn[:, 0:1])
                nc.scalar.mul(ts[1][:], pes[1][:], gn[:, 1:2])
                emit_corr()
                nc.scalar.mul(ts[2][:], pes[2][:], gn[:, 2:3])
                nc.scalar.mul(ts[3][:], pes[3][:], gn[:, 3:4])
                for c in range(KC):
                    for e in range(4, 8):
                        mm(e, c)
                    if g1_hook is not None:
                        g1_hook(c)
                emit_tail()
            elif skip_e012:
                # e0-e2 already ran, interleaved into tile 0's second half
                emit_gates()
                mm_expert(3)
                nc.scalar.mul(ts[0][:], pes[0][:, E:E + O], gn[:, 0:1])
                nc.scalar.mul(ts[1][:], pes[1][:], gn[:, 1:2])
                nc.scalar.mul(ts[2][:], pes[2][:], gn[:, 2:3])
                emit_corr()
                nc.scalar.mul(ts[3][:], pes[3][:], gn[:, 3:4])
                mm_expert(4)
                emit_tail()
            else:
                mm_expert(0)
                emit_gates()
                mm_expert(1)
                nc.scalar.mul(ts[0][:], pes[0][:, E:E + O], gn[:, 0:1])
                mm_expert(2)
                nc.scalar.mul(ts[1][:], pes[1][:], gn[:, 1:2])
                mm_expert(3)
                nc.scalar.mul(ts[2][:], pes[2][:], gn[:, 2:3])
                emit_corr()
                nc.scalar.mul(ts[3][:], pes[3][:], gn[:, 3:4])
                mm_expert(4)
                emit_tail()

        pes0 = alloc_pes(0)
        pes1 = alloc_pes(1)
        t1_g0 = [(e, c) for e in range(3) for c in range(KC)]

        def t1_hook(ci):
            # 3 of tile 1's e0-e2 matmuls per wB chunk (their banks are
            # freed by tile 0's ACT drain; e3+ would deadlock on a4)
            for e, c in t1_g0[3 * ci:3 * (ci + 1)]:
                lo = 0 if e == 0 else E + e * O
                nc.tensor.matmul(pes1[e][:], xr1[:, c, :],
                                 w_sb[c][:, lo:lo + pes1[e].shape[-1]],
                                 start=(c == 0), stop=(c == KC - 1))

        emit_tile(0, xr=xr0, pes_pre=pes0, g1_hook=t1_hook)
        emit_tile(1, xr=xr1, pes_pre=pes1, skip_e012=True)
        emit_tile(2, xr=xr2)
        for t in range(3, NT):
            emit_tile(t)

    nc.compile()
    return nc


def _prep_shared(w_gate, expert_w, expert_bias):
    # packed per-chunk weight rows: [gate(8) | expert0(300) | ... | expert7(300)]
    wg_c = w_gate.reshape(KC, 128, E).transpose(1, 0, 2)            # [128,6,8]
    we_c = expert_w.reshape(E, O, KC, 128).transpose(3, 2, 0, 1)    # [128,6,8,300]
    w_host = np.ascontiguousarray(np.concatenate(
        [wg_c, we_c.reshape(128, KC, E * O)], axis=2), dtype=np.float16)
    # negbw[e, l*O + o] = -sum_i expert_bias[e,l,i] * expert_w[e,o,i],
    # replicated at partition offsets 0/32/64/96 for the diagonal corr tiles
    nbw = -np.einsum('eli,eoi->elo', expert_bias, expert_w,
                     optimize=True).reshape(E, L * O).astype(np.float16)
    nbw_host = np.zeros((104, L * O), dtype=np.float16)
    for b in range(4):
        nbw_host[32 * b:32 * b + E] = nbw
    return w_host, nbw_host


def _make_in_maps(inputs):
    x = np.asarray(inputs["x"], dtype=np.float32)
    w_host, nbw_host = _prep_shared(
        np.asarray(inputs["w_gate"], dtype=np.float32),
        np.asarray(inputs["expert_w"], dtype=np.float32),
        np.asarray(inputs["expert_bias"], dtype=np.float32))
    in_maps = []
    for c in range(NCORES):
        xc = x[c * BC:(c + 1) * BC]                    # [64, 50, 768]
        xl = xc.transpose(1, 0, 2).reshape(TOK, D)     # l-major tokens
        xt = np.ascontiguousarray(
            xl.reshape(NT, P, KC, 128).transpose(0, 3, 2, 1),
            dtype=np.float16)
        in_maps.append({"xt": xt, "w": w_host, "nbw": nbw_host})
    return in_maps


def kernel(x, w_gate, expert_w, expert_bias):
    if "nc" not in _CACHE:
        _CACHE["nc"] = _build_nc()
    nc = _CACHE["nc"]

    in_maps = _make_in_maps({"x": x, "w_gate": w_gate, "expert_w": expert_w,
                             "expert_bias": expert_bias})

    res = bass_utils.run_bass_kernel_spmd(nc, in_maps,
                                          core_ids=list(range(NCORES)))

    outs = []
    for c in range(NCORES):
        oc = res.results[c]["out"].reshape(L, BC, O).transpose(1, 0, 2)
        outs.append(oc)
    return np.concatenate(outs, axis=0).astype(np.float32)


if __name__ == "__main__":
    rng = np.random.default_rng(0)
    inputs = {
        "x": rng.standard_normal((B, L, D), dtype=np.float32),
        "w_gate": (rng.standard_normal((D, E)) * 0.02).astype(np.float32),
        "expert_w": (rng.standard_normal((E, O, D)) * 0.02).astype(np.float32),
        "expert_bias": (rng.standard_normal((E, L, D)) * 0.02).astype(np.float32),
    }
    out = kernel(**inputs)
    print("out", out.shape, out.dtype, np.abs(out).mean())
